# revision 1
# baseline (speedup 1.0000x reference)
"""BertMoELayer (B=4, S=2048, H=768, F=3072, E=8, top-2) on 8 Trainium2 cores.

Expert-parallel: one expert per core; the host evaluates the router in fp32
only to DECIDE the shard assignment (which tokens go to which core, matching
jax.lax.top_k tie-breaking) and gathers each core's token subset. All numeric
computation of the layer runs on device:

  per core c, over its gathered tokens (capacity = max expert load, exact):
    logitsT = WrT^T @ xT                      (bf16 matmul, fp32 psum)
    w_c     = sigmoid(2*lc - m1 - m2)         (smooth top-2 softmax weight;
                                               m1/m2 = top-2 of this token's
                                               logits, lc = this expert's
                                               logit, == m1 or m2 bit-exactly)
    hT      = gelu(WiT^T @ xT + bi[c])        (bf16 matmul, fp32 psum)
    outT_c  = WoT-chunk^T @ hT + bo[c]        (bf16 matmul, fp32 psum;
                                               [h, token] orientation so the
                                               tail tokens cost proportionally)

The smooth w formula has no comparison cliffs: when bf16 logits reorder a
near-tie relative to the host's fp32 selection, the weight degrades
continuously (the swapped logits are equal to within the noise), so no
selection-consistency hazard exists between host and device. Logits are
bit-identical across cores (same k-chain accumulation order), so the two
selected cores' weights sum to exactly softmax's 1. The E-axis top-2
reductions run on GPSIMD (partition_all_reduce), the single Sigmoid runs
once per kernel (activation-table swaps against the gelus are the scalar
engine's enemy), and w leaves via one small DMA.

All tensors are HOST-PREPACKED into SBUF-partition-major layout ([128, ...]
with each partition's bytes contiguous in DRAM), so every DMA moves 3-9KB
contiguous lines per partition; 1KB-line views were measured at only
~150-200 GB/s per queue and starved the weight stream.

The host unshards by scatter-adding each core's expert rows scaled by its
device-computed routing weight.
"""

import numpy as np
import ml_dtypes

import bass_rust
import concourse.bass as bass
import concourse.tile as tile
from concourse import bacc, mybir
from concourse.bass_utils import run_bass_kernel_spmd

B, S, H, F, E = 4, 2048, 768, 3072, 8
T = B * S
N_CORES = 8
TOP_K = 2

P = 128          # SBUF partitions
KH = H // P      # 6   h-chunks
KF = F // P      # 24  f-chunks

F32 = mybir.dt.float32
BF16 = mybir.dt.bfloat16
BF16_NP = ml_dtypes.bfloat16

# wi column groups (in j units of 128): small first group so the first
# L1 chain's weight DMA lands quickly at startup
WI_GROUPS = (1, 4, 4, 4, 4, 4, 3)


def make_blocks(cap: int):
    """Token blocks: 512-token blocks (psum-bank-sized) with a 257..512
    tail; every block >= 256 tokens so L1 chains stay matmul-bound (a
    512-free bf16 matmul is 213ns vs ~97ns LDWEIGHTS, which walrus never
    accelerates with FWL)."""
    assert cap >= 512
    blocks = []
    rem = cap
    while rem > 768:
        blocks.append(512)
        rem -= 512
    if rem > 512:
        blocks.append(256)
        rem -= 256
    blocks.append(rem)
    assert sum(blocks) == cap
    assert all(b % 128 == 0 for b in blocks[:-1]) and blocks[-1] <= 512
    return blocks


def build_nc(cap: int):
    """Per-core program: router weight + dense expert FFN over `cap` tokens."""
    blocks = make_blocks(cap)
    nblk = len(blocks)

    # Bacc (not plain Bass): its compile() pass splits multi-wait instructions
    # into event-semaphore chains, which walrus requires (max 1 wait per inst).
    nc = bacc.Bacc(None)

    # All inputs prepacked on host to [128 partitions, contiguous bytes].
    xg = nc.declare_dram_parameter("xg", [P, KH * cap], BF16, isOutput=False)
    wiT = nc.declare_dram_parameter("wiT", [P, KH * F], BF16, isOutput=False)
    woT = nc.declare_dram_parameter("woT", [P, KF * H], BF16, isOutput=False)
    wrT = nc.declare_dram_parameter("wrT", [P, KH * E], BF16, isOutput=False)
    bi = nc.declare_dram_parameter("bi", [P, KF], F32, isOutput=False)
    boT = nc.declare_dram_parameter("boT", [P, KH], F32, isOutput=False)
    esel = nc.declare_dram_parameter("esel", [E, 1], F32, isOutput=False)
    # transposed output: token t0+t of block ib lives at [p=h%128, KH*t0 +
    # k*b + t] per h-chunk k (block-major, like xg). Unweighted expert rows;
    # the device-computed routing weights stream out via out_w and the host
    # applies them during the scatter-add unshard.
    out = nc.declare_dram_parameter("out", [P, KH * cap], F32, isOutput=True)
    out_w = nc.declare_dram_parameter("out_w", [1, cap], F32, isOutput=True)

    # j (0..23) -> (wi group tile index, local column slice)
    j_map = []
    for gi, gw in enumerate(WI_GROUPS):
        for jj in range(gw):
            j_map.append((gi, jj))
    # wi group g covers columns [goff[g]*128, (goff[g]+gw)*128)
    goff = [sum(WI_GROUPS[:g]) for g in range(len(WI_GROUPS))]
    # L1 chains consume j's in GROUP-ARRIVAL order; with g0 on the scalar
    # ring, g1-g4+g6 streaming in order on sync, and g5 paced on scalar,
    # arrival order is natural.
    GROUP_ORDER = (0, 1, 2, 3, 4, 5, 6)
    # k (contraction chunk) order matches x0's three-piece arrival: k0-1
    # first on sync, k4-5 on scalar, k2-3 second on sync. PSUM accumulation
    # is order-free; later blocks' x arrives whole so order is irrelevant.
    K_ORDER = (0, 1, 4, 5, 2, 3)
    j_order = [
        goff[g] + jj for g in GROUP_ORDER for jj in range(WI_GROUPS[g])
    ]

    with tile.TileContext(nc) as tc:
        with (
            tc.tile_pool(name="weights", bufs=1) as wpool,
            tc.tile_pool(name="xin", bufs=3) as xpool,
            tc.tile_pool(name="hbuf", bufs=2) as hpool,
            tc.tile_pool(name="obuf", bufs=2) as opool,
            tc.tile_pool(name="router", bufs=1) as rpool,
            tc.tile_pool(name="psum_h", bufs=4, space="PSUM") as ph_pool,
            tc.tile_pool(name="psum_o", bufs=4, space="PSUM") as po_pool,
        ):
            # ---- preamble DMAs, split across the two HWDGE rings (sync +
            # scalar) in consumption order. xg block ib lives at flat offset
            # KH*t0 (block-major host packing -> 6KB lines / partition). ----
            def x_dma(eng, xt, t0, b):
                eng.dma_start(
                    out=xt,
                    in_=xg[:, KH * t0 : KH * (t0 + b)].rearrange(
                        "p (k t) -> p k t", k=KH
                    ),
                )

            def wig_dma(eng, wt, g):
                a = KH * P * goff[g]
                w = KH * P * WI_GROUPS[g]
                eng.dma_start(
                    out=wt,
                    in_=wiT[:, a : a + w].rearrange("p (k c) -> p k c", k=KH),
                )

            # Engine budget note: DMA_DIRECT2D injection costs ~0.7-1.9us of
            # the ISSUING engine's time. The scalar engine must be free for
            # gelus by ~12us, so it only issues the three critical early
            # tiles; everything else goes on sync (SP ring) or gpsimd (SW
            # ring). Per-ring HBM bandwidth is ~200-250 GB/s concurrent.
            x_tiles = {}
            b0 = blocks[0]
            x0_bf = xpool.tile([P, KH, b0], BF16, tag="xb", name="x0_bf")
            x_tiles[0] = x0_bf

            wrT_sb = wpool.tile([P, KH, E], BF16)
            wi_groups = [
                wpool.tile(
                    [P, KH, gw * P], BF16, tag=f"wig{gi}", name=f"wig{gi}"
                )
                for gi, gw in enumerate(WI_GROUPS)
            ]
            bi_sb = wpool.tile([P, KF], F32)
            woT_sb = wpool.tile([P, KF, H], BF16)

            def wo_dma(eng, g, n):
                eng.dma_start(
                    out=woT_sb[:, g : g + n, :],
                    in_=woT[:, g * H : (g + n) * H].rearrange(
                        "p (j h) -> p j h", j=n
                    ),
                )

            # Ring assignment tuned to measured rates (sync ~200 GB/s,
            # scalar ~117, gpsimd ~79) and need times. Every DMA_DIRECT2D
            # costs ~0.7-1.9us of the issuing engine's time, so the scalar
            # engine (gelus from ~12us) gets only 3 early injections;
            # wig5/woT3 are paced from inside block 0's j-loop.
            # x0 split in k-pair thirds across all three rings: the first
            # transfer on each ring runs at only ~90 GB/s, so parallelism is
            # the only way to get the first-matmul gate down
            def x0_dma(eng, k0, k1):
                eng.dma_start(
                    out=x0_bf[:, k0:k1, :],
                    in_=xg[:, k0 * b0 : k1 * b0].rearrange(
                        "p (k t) -> p k t", k=k1 - k0
                    ),
                )

            nc.sync.dma_start(
                out=wrT_sb, in_=wrT.rearrange("p (k e) -> p k e", k=KH)
            )
            x0_dma(nc.sync, 0, 2)
            wig_dma(nc.scalar, wi_groups[0], 0)
            x0_dma(nc.scalar, 4, 6)
            x0_dma(nc.sync, 2, 4)
            # one-hot expert selector as a per-partition scalar column
            nc.scalar.dma_start(out=bi_sb, in_=bi[:, :])
            esel_col = wpool.tile([E, 1], F32)
            nc.scalar.dma_start(out=esel_col, in_=esel[:, :])
            boT_sb = wpool.tile([P, KH], F32)
            nc.scalar.dma_start(out=boT_sb, in_=boT[:, :])
            # the wi/wo bulk rides the sync ring BEHIND x0 in need order, so
            # x0 (the first-matmul gate) isn't bandwidth-shared against it
            wig_dma(nc.sync, wi_groups[1], 1)
            wig_dma(nc.sync, wi_groups[2], 2)
            wig_dma(nc.sync, wi_groups[3], 3)
            wig_dma(nc.sync, wi_groups[4], 4)
            wig_dma(nc.sync, wi_groups[6], 6)
            wo_dma(nc.sync, 0, 6)
            wo_dma(nc.sync, 6, 6)
            wo_dma(nc.gpsimd, 12, 6)
            wo_dma(nc.sync, 18, 6)
            # sigmoid argument (2*lc - m1 - m2) for the whole core, written
            # incrementally per block; one Sigmoid + one DMA at the end (the
            # host applies the weights during unshard)
            d_all = wpool.tile([E, cap], F32)
            w_all = wpool.tile([E, cap], F32)

            def router_logits(x_bf, b):
                # logitsT [E, b] via the same bf16 x the FFN uses; fp32 psum.
                # Lives in the L1 chain psum pool (saves a PSUM bank so the
                # L2 pool can triple-buffer).
                pslT = ph_pool.tile([E, b], F32, tag="ph")
                for ki, k in enumerate(K_ORDER):
                    nc.tensor.matmul(
                        pslT,
                        lhsT=wrT_sb[:, k, :],
                        rhs=x_bf[:, k, :],
                        start=(ki == 0),
                        stop=(ki == KH - 1),
                    )
                # psum -> sbuf copy on the SCALAR engine: the DVE queue is
                # in-order and its tail (previous block's L2 epilogue ops,
                # which wait on L2 psums) would stall this copy and with it
                # the next block's transposes on the PE.
                lgT_sb = rpool.tile([E, b], F32, tag="lgT", bufs=2)
                nc.scalar.activation(
                    lgT_sb, pslT, mybir.ActivationFunctionType.Copy
                )
                return lgT_sb

            def router_weights(lgT_sb, b, t0):
                """Sigmoid argument for the top-2 softmax weight, entirely in
                the natural [E, tokens] layout: the E-axis reductions run as
                GPSIMD partition_all_reduce, so no PE transposes and no
                deadline — w is only read by the final out_w DMA.

                lc (this expert's logit) is bit-exactly m1 or m2, so
                w = 1/(exp(m1-lc)+exp(m2-lc)) = sigmoid(2*lc - m1 - m2)."""
                m1t = rpool.tile([E, b], F32, tag="m1")
                nc.gpsimd.partition_all_reduce(
                    m1t, lgT_sb, channels=E, reduce_op=bass_rust.ReduceOp.max
                )
                # mask out the argmax (ties: both masked, m2 = 3rd logit)
                ge = rpool.tile([E, b], F32, tag="ge")
                nc.vector.tensor_tensor(
                    ge, lgT_sb, m1t, op=mybir.AluOpType.is_ge
                )
                mk = rpool.tile([E, b], F32, tag="mk")
                nc.vector.scalar_tensor_tensor(
                    mk, in0=ge, scalar=-1e30, in1=lgT_sb,
                    op0=mybir.AluOpType.mult, op1=mybir.AluOpType.add,
                )
                m2t = rpool.tile([E, b], F32, tag="m2")
                nc.gpsimd.partition_all_reduce(
                    m2t, mk, channels=E, reduce_op=bass_rust.ReduceOp.max
                )
                # this core's logit row, replicated to all E partitions
                lce = rpool.tile([E, b], F32, tag="lce")
                nc.vector.tensor_scalar_mul(lce, lgT_sb, scalar1=esel_col)
                lct = rpool.tile([E, b], F32, tag="lc")
                nc.gpsimd.partition_all_reduce(
                    lct, lce, channels=E, reduce_op=bass_rust.ReduceOp.add
                )
                m12 = rpool.tile([E, b], F32, tag="m12")
                nc.vector.tensor_tensor(m12, m1t, m2t, op=mybir.AluOpType.add)
                nc.vector.scalar_tensor_tensor(
                    d_all[:, t0 : t0 + b], in0=lct, scalar=2.0, in1=m12,
                    op0=mybir.AluOpType.mult, op1=mybir.AluOpType.subtract,
                )

            t0 = 0
            for ib, b in enumerate(blocks):
                last_blk = ib == nblk - 1

                x_bf = x_tiles.pop(ib)
                # prefetch next block's x; issued here (not in the preamble) so
                # it doesn't compete with the wi/wo weight stream at startup
                if ib + 1 < nblk:
                    bn = blocks[ib + 1]
                    x_next = xpool.tile([P, KH, bn], BF16, tag="xb", name="x_next")
                    x_tiles[ib + 1] = x_next
                    x_dma(nc.sync, x_next, t0 + b, bn)

                # ---- layer 1: hT[f, t] = gelu(WiT^T @ xT + bi), with the
                # router work interleaved between the dense j-chains so the
                # PE activity stays dense. Router logits go right after the
                # j=0 chain: the extra PE time buys slack for the wi-group
                # weight stream at startup. ----
                hT = hpool.tile([P, KF, b], BF16, tag="hT")
                for idx, j in enumerate(j_order):
                    gi, jj = j_map[j]
                    ps = ph_pool.tile([P, b], F32, tag="ph")
                    wig = wi_groups[gi]
                    for ki, k in enumerate(K_ORDER):
                        nc.tensor.matmul(
                            ps,
                            lhsT=wig[:, k, jj * P : (jj + 1) * P],
                            rhs=x_bf[:, k, :],
                            start=(ki == 0),
                            stop=(ki == KH - 1),
                        )
                    nc.scalar.activation(
                        out=hT[:, j, :],
                        in_=ps,
                        func=mybir.ActivationFunctionType.Gelu,
                        bias=bi_sb[:, j : j + 1],
                        scale=1.0,
                    )
                    if idx == 0:
                        lgT_sb = router_logits(x_bf, b)
                    elif idx == 1:
                        router_weights(lgT_sb, b, t0)
                    if ib == 0 and idx == 4:
                        # paced late load: injected between gelus so its
                        # HBM traffic starts after the startup crunch
                        wig_dma(nc.scalar, wi_groups[5], 5)

                # ---- layer 2 (transposed): outT[h, t] = WoT-chunk^T @ hT + bo.
                # Tokens are the matmul free dim, so a partial tail tile
                # costs proportionally (no half-empty 128-token sweep), and
                # bo is a per-partition scalar in this orientation. ----
                o_blkT = opool.tile([P, KH, b], F32, tag="os")
                for c in range(KH):
                    pc = po_pool.tile([P, b], F32, tag="po")
                    for j in range(KF):
                        nc.tensor.matmul(
                            pc,
                            lhsT=woT_sb[:, j, c * P : (c + 1) * P],
                            rhs=hT[:, j, :],
                            start=(j == 0),
                            stop=(j == KF - 1),
                        )
                    nc.vector.tensor_scalar(
                        o_blkT[:, c, :], pc, scalar1=boT_sb[:, c : c + 1],
                        scalar2=None, op0=mybir.AluOpType.add,
                    )
                    if last_blk:
                        # per-chunk writes on the last block, alternating
                        # rings: the final DMA after the last epilogue is one
                        # 2KB-line chunk and the injects don't serialize on
                        # one engine (shorter teardown tail)
                        eng = nc.sync if c % 2 == 0 else nc.scalar
                        eng.dma_start(
                            out=out[:, KH * t0 + c * b : KH * t0 + (c + 1) * b],
                            in_=o_blkT[:, c, :],
                        )
                if not last_blk:
                    nc.sync.dma_start(
                        out=out[:, KH * t0 : KH * (t0 + b)].rearrange(
                            "p (k t) -> p k t", k=KH
                        ),
                        in_=o_blkT,
                    )
                t0 += b
                if last_blk:
                    # pin d_all behind the last block's first L2 epilogue so
                    # the (ASAP-scheduled) Sigmoid — the kernel's only
                    # Gelu-table eviction — lands in the gelu-free last L2
                    # window, not between gelus
                    nc.vector.scalar_tensor_tensor(
                        d_all[:, 0:1], in0=o_blkT[0:E, 0, 0:1], scalar=0.0,
                        in1=d_all[:, 0:1], op0=mybir.AluOpType.mult,
                        op1=mybir.AluOpType.add,
                    )

            # one sigmoid + one routing-weight writeback for the whole core
            # (token order = slot order)
            nc.scalar.activation(
                w_all, d_all, mybir.ActivationFunctionType.Sigmoid
            )
            nc.scalar.dma_start(out=out_w[:, :], in_=w_all[0:1, :])

    nc.compile()
    return nc


_NC_CACHE: dict = {}


def _get_nc(cap: int):
    if cap not in _NC_CACHE:
        _NC_CACHE[cap] = build_nc(cap)
    return _NC_CACHE[cap]


def _ensure_axon_hooks_module():
    """run_bass_kernel_spmd(trace=True) (e.g. via env BASS_TRACE=1) imports
    antenv.axon_hooks, which some images lack even though the boot code that
    would register the NTFF hook is present. Provide the module and register
    the real hook when available so tracing works instead of crashing."""
    try:
        import antenv.axon_hooks  # noqa: F401

        return
    except ImportError:
        pass
    try:
        import sys
        import types

        import antenv  # noqa: F401

        mod = types.ModuleType("antenv.axon_hooks")
        state = {"hook": None}
        mod.set_axon_ntff_profile_hook = lambda h: state.__setitem__("hook", h)
        mod.get_axon_ntff_profile_hook = lambda: state["hook"]
        try:
            from trn_agent_boot.trn_boot import _ntff_profile_via_ctypes

            mod.set_axon_ntff_profile_hook(
                _ntff_profile_via_ctypes("/opt/axon/libaxon_pjrt.so")
            )
        except Exception:
            pass
        sys.modules["antenv.axon_hooks"] = mod
    except Exception:
        pass


def _shard_tokens(xf, Wr):
    """Host-side sharding function: top-2 expert index per token (matches
    jax.lax.top_k tie-breaking: lowest index wins on ties)."""
    logits = xf.astype(np.float32) @ np.asarray(Wr, np.float32).T  # [T, E]
    i1 = np.argmax(logits, axis=1)
    l2 = logits.copy()
    l2[np.arange(len(i1)), i1] = -np.inf
    i2 = np.argmax(l2, axis=1)
    tokens = np.arange(logits.shape[0])
    tok_lists = []
    for c in range(N_CORES):
        tok_lists.append(np.concatenate([tokens[i1 == c], tokens[i2 == c]]))
    return tok_lists


def _pack_kpf(a2d, k):
    """[k*128, N] row-major -> [128, k*N] partition-major (k-major per row)."""
    kk, n = a2d.shape
    assert kk == k * P
    return np.ascontiguousarray(
        a2d.reshape(k, P, n).transpose(1, 0, 2).reshape(P, k * n)
    )


def _pack_wi_groups(wiT2d):
    """[H, F] -> [128, KH*F] GROUP-major: each wi column group's
    [KH, group_cols] block is contiguous per partition."""
    v = wiT2d.reshape(KH, P, F)
    parts = []
    c0 = 0
    for gw in WI_GROUPS:
        parts.append(
            v[:, :, c0 : c0 + gw * P].transpose(1, 0, 2).reshape(P, KH * gw * P)
        )
        c0 += gw * P
    return np.ascontiguousarray(np.concatenate(parts, axis=1))


def kernel(x, Wr, Wi, bi, Wo, bo, _trace=False):
    x = np.asarray(x)
    xf = x.reshape(-1, H).astype(np.float32)
    tok_lists = _shard_tokens(xf, Wr)
    cap = max(512, max(len(tl) for tl in tok_lists))
    blocks = make_blocks(cap)

    xT = np.ascontiguousarray(xf.T).astype(BF16_NP)  # [H, T] bf16
    wrT_p = _pack_kpf(
        np.ascontiguousarray(np.asarray(Wr, np.float32).T).astype(BF16_NP), KH
    )
    bi_full = np.asarray(bi, np.float32)
    bo_full = np.asarray(bo, np.float32)

    in_maps = []
    for c in range(N_CORES):
        tl = tok_lists[c]
        xg = np.zeros((H, cap), dtype=BF16_NP)
        xg[:, : len(tl)] = xT[:, tl]
        # block-major packing: [128, sum_b KH*b], block ib at offset KH*t0
        xg_k = xg.reshape(KH, P, cap)
        xg_p = np.empty((P, KH * cap), dtype=BF16_NP)
        t0 = 0
        for b in blocks:
            xg_p[:, KH * t0 : KH * (t0 + b)] = (
                xg_k[:, :, t0 : t0 + b].transpose(1, 0, 2).reshape(P, KH * b)
            )
            t0 += b
        sel = np.zeros((E, 1), np.float32)
        sel[c, 0] = 1.0
        in_maps.append(
            {
                "xg": xg_p,
                "wiT": _pack_wi_groups(
                    np.asarray(Wi[c], np.float32).T.astype(BF16_NP)
                ),
                "woT": _pack_kpf(
                    np.ascontiguousarray(np.asarray(Wo[c], np.float32).T).astype(
                        BF16_NP
                    ),
                    KF,
                ),
                "wrT": wrT_p,
                "bi": _pack_kpf(bi_full[c].reshape(F, 1), KF).reshape(P, KF),
                "boT": _pack_kpf(bo_full[c].reshape(H, 1), KH).reshape(P, KH),
                "esel": sel,
            }
        )

    _ensure_axon_hooks_module()
    nc = _get_nc(cap)
    res = run_bass_kernel_spmd(
        nc, in_maps, core_ids=list(range(N_CORES)), trace=_trace
    )

    # Unshard: scatter-add each core's expert rows scaled by its
    # device-computed routing weight (the combine's indexed accumulation
    # runs on host either way; w itself was computed on device).
    out = np.zeros((T, H), dtype=np.float32)
    for c in range(N_CORES):
        tl = tok_lists[c]
        n = len(tl)
        # out param is [128, KH*cap] block-major: token t0+t of block ib at
        # [p, KH*t0 + k*b + t] -> rows h = k*128+p
        o = res.results[c]["out"]  # [P, KH*cap]
        o_rows = np.empty((n, H), dtype=np.float32)
        t0 = 0
        for b in blocks:
            if t0 >= n:
                break
            m = min(b, n - t0)
            blk = o[:, KH * t0 : KH * (t0 + b)].reshape(P, KH, b)
            # [P, KH, b] -> tokens t0..t0+m, features h=k*128+p
            o_rows[t0 : t0 + m] = blk[:, :, :m].transpose(2, 1, 0).reshape(m, H)
            t0 += b
        # w: [1, cap] in token-slot order
        w = res.results[c]["out_w"][0, :n]
        out[tl] += w[:, None] * o_rows
    out = out.reshape(x.shape)
    if _trace:
        return out, res
    return out



# revision 2
# speedup vs baseline: 1.0089x; 1.0089x over previous
"""BertMoELayer (B=4, S=2048, H=768, F=3072, E=8, top-2) on 8 Trainium2 cores.

Expert-parallel: one expert per core. The host evaluates the router in fp32
(it must anyway, to decide the shard assignment matching jax.lax.top_k
tie-breaking) and also produces the top-2 softmax weights there — they are
O(T) scalars and bit-match the reference combine. The device runs the dense
FFN only:

  per core c, over its gathered tokens (capacity = max expert load, exact):
    hT      = gelu(WiT^T @ xT + bi[c])        (bf16 matmul, fp32 psum)
    outT_c  = WoT-chunk^T @ hT + bo[c]        (bf16 matmul, fp32 psum, bf16 out;
                                               [h, token] orientation so the
                                               tail tokens cost proportionally)

The host unshards by scatter-adding each core's expert rows scaled by its
fp32 routing weight.

Startup is the only non-roofline phase, so the DMA schedule is built around
the measured ring behavior: per-queue throughput ramps with per-partition
line size (2KB lines ~50 GB/s early, 6-18KB lines 230+ GB/s), the event-
semaphore pool that tracks DMA completion is small (~8) so injects beyond it
serialize on earlier completions, and each DMA_DIRECT2D costs ~0.7us of the
issuing engine. Hence:
  - x block 0 goes on the gpsimd ring (fastest ramp, nothing else to do) in
    two k-pieces so the first L1 matmul can start ~3us into the body;
  - wi streams in ramping group sizes (1,2,3,4 cols*128 on sync; 6,8 on
    gpsimd behind x0) so chain j's weights always land just ahead of it;
  - wo is packed C-MAJOR (output-chunk-major) and split scalar(c0-1)/
    gpsimd(c2-5), so layer 2 can start on a partial wo stream;
  - bi+bo ride one tiny scalar-ring DMA; scalar issues only 2 injects and
    is then free for the gelu chain from ~3us.

All tensors are HOST-PREPACKED into SBUF-partition-major layout ([128, ...]
with each partition's bytes contiguous in DRAM): every DMA moves 3-24KB
contiguous lines per partition.
"""

import numpy as np
import ml_dtypes

import concourse.bass as bass
import concourse.tile as tile
from concourse import bacc, mybir
from concourse.bass_utils import run_bass_kernel_spmd

B, S, H, F, E = 4, 2048, 768, 3072, 8
T = B * S
N_CORES = 8

P = 128          # SBUF partitions
KH = H // P      # 6   h-chunks
KF = F // P      # 24  f-chunks

F32 = mybir.dt.float32
BF16 = mybir.dt.bfloat16
BF16_NP = ml_dtypes.bfloat16

# wi column groups (in j units of 128): ramping sizes so group g+1's DMA
# completes just before the L1 chains finish consuming group g at startup
WI_GROUPS = (1, 2, 3, 4, 6, 8)


def make_blocks(cap: int):
    """Token blocks: 512-token blocks (psum-bank-sized) with a 257..512
    tail; every block >= 256 tokens so L1 chains stay matmul-bound (a
    512-free bf16 matmul is 213ns vs ~97ns LDWEIGHTS)."""
    assert cap >= 512
    blocks = []
    rem = cap
    while rem > 768:
        blocks.append(512)
        rem -= 512
    if rem > 512:
        blocks.append(256)
        rem -= 256
    blocks.append(rem)
    assert sum(blocks) == cap
    assert all(b % 128 == 0 for b in blocks[:-1]) and blocks[-1] <= 512
    return blocks


def build_nc(cap: int):
    """Per-core program: dense expert FFN over `cap` tokens."""
    blocks = make_blocks(cap)
    nblk = len(blocks)

    # Bacc (not plain Bass): its compile() pass splits multi-wait instructions
    # into event-semaphore chains, which walrus requires (max 1 wait per inst).
    nc = bacc.Bacc(None)

    # All inputs prepacked on host to [128 partitions, contiguous bytes].
    xg = nc.declare_dram_parameter("xg", [P, KH * cap], BF16, isOutput=False)
    wiT = nc.declare_dram_parameter("wiT", [P, KH * F], BF16, isOutput=False)
    # c-major: [P, c(6), j(24), col(128)] flattened
    woT = nc.declare_dram_parameter("woT", [P, KH * KF * P], BF16, isOutput=False)
    bibo = nc.declare_dram_parameter("bibo", [P, KF + KH], F32, isOutput=False)
    # transposed output: token t0+t of block ib lives at [p=h%128, KH*t0 +
    # k*b + t] per h-chunk k (block-major, like xg). Unweighted expert rows
    # in bf16; the host applies the fp32 routing weights during scatter-add.
    out = nc.declare_dram_parameter("out", [P, KH * cap], BF16, isOutput=True)

    # j (0..23) -> (wi group tile index, local column slice)
    j_map = []
    for gi, gw in enumerate(WI_GROUPS):
        for jj in range(gw):
            j_map.append((gi, jj))
    goff = [sum(WI_GROUPS[:g]) for g in range(len(WI_GROUPS))]

    with tile.TileContext(nc) as tc:
        with (
            tc.tile_pool(name="weights", bufs=1) as wpool,
            tc.tile_pool(name="xin", bufs=3) as xpool,
            tc.tile_pool(name="hbuf", bufs=2) as hpool,
            tc.tile_pool(name="obuf", bufs=2) as opool,
            tc.tile_pool(name="psum_h", bufs=4, space="PSUM") as ph_pool,
            tc.tile_pool(name="psum_o", bufs=4, space="PSUM") as po_pool,
        ):
            def x_dma(eng, xt, t0, b):
                eng.dma_start(
                    out=xt,
                    in_=xg[:, KH * t0 : KH * (t0 + b)].rearrange(
                        "p (k t) -> p k t", k=KH
                    ),
                )

            def wig_dma(eng, wt, g):
                a = KH * P * goff[g]
                w = KH * P * WI_GROUPS[g]
                eng.dma_start(
                    out=wt,
                    in_=wiT[:, a : a + w].rearrange("p (k c) -> p k c", k=KH),
                )

            b0 = blocks[0]
            x_tiles = {}
            x0_bf = xpool.tile([P, KH, b0], BF16, tag="xb", name="x0_bf")
            x_tiles[0] = x0_bf
            wi_groups = [
                wpool.tile(
                    [P, KH, gw * P], BF16, tag=f"wig{gi}", name=f"wig{gi}"
                )
                for gi, gw in enumerate(WI_GROUPS)
            ]
            bibo_sb = wpool.tile([P, KF + KH], F32)
            wo_sb = wpool.tile([P, KH, KF * P], BF16)

            def wo_dma(eng, c0, c1):
                eng.dma_start(
                    out=wo_sb[:, c0:c1, :],
                    in_=woT[:, c0 * KF * P : c1 * KF * P].rearrange(
                        "p (c n) -> p c n", c=c1 - c0
                    ),
                )

            # ---- preamble DMAs; order tuned to need times (see module doc).
            # x0 in two k-pieces on gpsimd: first L1 matmuls gate only on k0-2.
            nc.gpsimd.dma_start(
                out=x0_bf[:, 0:3, :],
                in_=xg[:, 0 : 3 * b0].rearrange("p (k t) -> p k t", k=3),
            )
            nc.gpsimd.dma_start(
                out=x0_bf[:, 3:6, :],
                in_=xg[:, 3 * b0 : 6 * b0].rearrange("p (k t) -> p k t", k=3),
            )
            wig_dma(nc.sync, wi_groups[0], 0)
            wig_dma(nc.sync, wi_groups[1], 1)
            wig_dma(nc.sync, wi_groups[2], 2)
            wig_dma(nc.sync, wi_groups[3], 3)
            nc.scalar.dma_start(out=bibo_sb, in_=bibo[:, :])
            wo_dma(nc.scalar, 0, 2)
            wig_dma(nc.gpsimd, wi_groups[4], 4)
            wig_dma(nc.gpsimd, wi_groups[5], 5)
            if nblk > 1:
                b1 = blocks[1]
                x1_bf = xpool.tile([P, KH, b1], BF16, tag="xb", name="x1_bf")
                x_tiles[1] = x1_bf
                x_dma(nc.sync, x1_bf, b0, b1)
            if nblk > 2:
                b2 = blocks[2]
                x2_bf = xpool.tile([P, KH, b2], BF16, tag="xb", name="x2_bf")
                x_tiles[2] = x2_bf
                x_dma(nc.sync, x2_bf, b0 + b1, b2)
            wo_dma(nc.gpsimd, 2, 6)

            t0 = 0
            for ib, b in enumerate(blocks):
                last_blk = ib == nblk - 1

                x_bf = x_tiles.pop(ib)
                # prefetch block ib+2's x (ib, ib+1, ib+2 resident; bufs=3)
                if ib >= 1 and ib + 2 < nblk:
                    bn = blocks[ib + 2]
                    x_next = xpool.tile(
                        [P, KH, bn], BF16, tag="xb", name=f"x{ib + 2}_bf"
                    )
                    x_tiles[ib + 2] = x_next
                    x_dma(nc.sync, x_next, sum(blocks[: ib + 2]), bn)

                # ---- layer 1: hT[f, t] = gelu(WiT^T @ xT + bi) ----
                hT = hpool.tile([P, KF, b], BF16, tag="hT")
                for j in range(KF):
                    gi, jj = j_map[j]
                    ps = ph_pool.tile([P, b], F32, tag="ph")
                    wig = wi_groups[gi]
                    for k in range(KH):
                        nc.tensor.matmul(
                            ps,
                            lhsT=wig[:, k, jj * P : (jj + 1) * P],
                            rhs=x_bf[:, k, :],
                            start=(k == 0),
                            stop=(k == KH - 1),
                        )
                    nc.scalar.activation(
                        out=hT[:, j, :],
                        in_=ps,
                        func=mybir.ActivationFunctionType.Gelu,
                        bias=bibo_sb[:, j : j + 1],
                        scale=1.0,
                    )

                # ---- layer 2 (transposed): outT[h, t] = WoT-chunk^T @ hT + bo.
                # Tokens are the matmul free dim, so a partial tail tile
                # costs proportionally, and bo is a per-partition scalar. ----
                o_blkT = opool.tile([P, KH, b], BF16, tag="os")
                for c in range(KH):
                    pc = po_pool.tile([P, b], F32, tag="po")
                    for j in range(KF):
                        nc.tensor.matmul(
                            pc,
                            lhsT=wo_sb[:, c, j * P : (j + 1) * P],
                            rhs=hT[:, j, :],
                            start=(j == 0),
                            stop=(j == KF - 1),
                        )
                    nc.vector.tensor_scalar(
                        o_blkT[:, c, :], pc,
                        scalar1=bibo_sb[:, KF + c : KF + c + 1],
                        scalar2=None, op0=mybir.AluOpType.add,
                    )
                    if last_blk:
                        # per-chunk writes on the last block, alternating
                        # rings: the final DMA after the last epilogue is one
                        # small chunk and the injects don't serialize on one
                        # engine (shorter teardown tail)
                        eng = nc.sync if c % 2 == 0 else nc.scalar
                        eng.dma_start(
                            out=out[:, KH * t0 + c * b : KH * t0 + (c + 1) * b],
                            in_=o_blkT[:, c, :],
                        )
                if not last_blk:
                    nc.sync.dma_start(
                        out=out[:, KH * t0 : KH * (t0 + b)].rearrange(
                            "p (k t) -> p k t", k=KH
                        ),
                        in_=o_blkT,
                    )
                t0 += b

    nc.compile()
    return nc


_NC_CACHE: dict = {}


def _get_nc(cap: int):
    if cap not in _NC_CACHE:
        _NC_CACHE[cap] = build_nc(cap)
    return _NC_CACHE[cap]


def _ensure_axon_hooks_module():
    """run_bass_kernel_spmd(trace=True) (e.g. via env BASS_TRACE=1) imports
    antenv.axon_hooks, which some images lack even though the boot code that
    would register the NTFF hook is present. Provide the module and register
    the real hook when available so tracing works instead of crashing."""
    try:
        import antenv.axon_hooks  # noqa: F401

        return
    except ImportError:
        pass
    try:
        import sys
        import types

        import antenv  # noqa: F401

        mod = types.ModuleType("antenv.axon_hooks")
        state = {"hook": None}
        mod.set_axon_ntff_profile_hook = lambda h: state.__setitem__("hook", h)
        mod.get_axon_ntff_profile_hook = lambda: state["hook"]
        try:
            from trn_agent_boot.trn_boot import _ntff_profile_via_ctypes

            mod.set_axon_ntff_profile_hook(
                _ntff_profile_via_ctypes("/opt/axon/libaxon_pjrt.so")
            )
        except Exception:
            pass
        sys.modules["antenv.axon_hooks"] = mod
    except Exception:
        pass


def _route(xf, Wr):
    """Host router in fp32: top-2 expert indices (matching jax.lax.top_k
    tie-breaking: lowest index wins) and softmax weights over the top-2."""
    logits = xf.astype(np.float32) @ np.asarray(Wr, np.float32).T  # [T, E]
    i1 = np.argmax(logits, axis=1)
    l2 = logits.copy()
    rows = np.arange(len(i1))
    l2[rows, i1] = -np.inf
    i2 = np.argmax(l2, axis=1)
    m1 = logits[rows, i1]
    m2 = l2[rows, i2]
    e = np.exp(m2 - m1)
    w1 = 1.0 / (1.0 + e)
    w2 = e / (1.0 + e)
    tokens = np.arange(logits.shape[0])
    tok_lists, w_lists = [], []
    for c in range(N_CORES):
        tok_lists.append(np.concatenate([tokens[i1 == c], tokens[i2 == c]]))
        w_lists.append(np.concatenate([w1[i1 == c], w2[i2 == c]]))
    return tok_lists, w_lists


def _pack_kpf(a2d, k):
    """[k*128, N] row-major -> [128, k*N] partition-major (k-major per row)."""
    kk, n = a2d.shape
    assert kk == k * P
    return np.ascontiguousarray(
        a2d.reshape(k, P, n).transpose(1, 0, 2).reshape(P, k * n)
    )


def _pack_wi_groups(wiT2d):
    """[H, F] -> [128, KH*F] GROUP-major: each wi column group's
    [KH, group_cols] block is contiguous per partition."""
    v = wiT2d.reshape(KH, P, F)
    parts = []
    c0 = 0
    for gw in WI_GROUPS:
        parts.append(
            v[:, :, c0 : c0 + gw * P].transpose(1, 0, 2).reshape(P, KH * gw * P)
        )
        c0 += gw * P
    return np.ascontiguousarray(np.concatenate(parts, axis=1))


def _pack_wo_cmajor(woT2d):
    """[F, H] -> [128, KH*KF*128] c-major: per partition p (=f%128), layout
    [c][j][col] with element = WoT[j*128+p, c*128+col]."""
    v = woT2d.reshape(KF, P, KH, P)  # [j, p, c, col]
    return np.ascontiguousarray(
        v.transpose(1, 2, 0, 3).reshape(P, KH * KF * P)
    )


def kernel(x, Wr, Wi, bi, Wo, bo, _trace=False):
    x = np.asarray(x)
    xf = x.reshape(-1, H).astype(np.float32)
    tok_lists, w_lists = _route(xf, Wr)
    cap = max(512, max(len(tl) for tl in tok_lists))
    blocks = make_blocks(cap)

    xT = np.ascontiguousarray(xf.T).astype(BF16_NP)  # [H, T] bf16
    bi_full = np.asarray(bi, np.float32)
    bo_full = np.asarray(bo, np.float32)

    in_maps = []
    for c in range(N_CORES):
        tl = tok_lists[c]
        xg = np.zeros((H, cap), dtype=BF16_NP)
        xg[:, : len(tl)] = xT[:, tl]
        # block-major packing: [128, sum_b KH*b], block ib at offset KH*t0
        xg_k = xg.reshape(KH, P, cap)
        xg_p = np.empty((P, KH * cap), dtype=BF16_NP)
        t0 = 0
        for b in blocks:
            xg_p[:, KH * t0 : KH * (t0 + b)] = (
                xg_k[:, :, t0 : t0 + b].transpose(1, 0, 2).reshape(P, KH * b)
            )
            t0 += b
        bibo_c = np.concatenate(
            [
                _pack_kpf(bi_full[c].reshape(F, 1), KF).reshape(P, KF),
                _pack_kpf(bo_full[c].reshape(H, 1), KH).reshape(P, KH),
            ],
            axis=1,
        )
        in_maps.append(
            {
                "xg": xg_p,
                "wiT": _pack_wi_groups(
                    np.asarray(Wi[c], np.float32).T.astype(BF16_NP)
                ),
                "woT": _pack_wo_cmajor(
                    np.ascontiguousarray(np.asarray(Wo[c], np.float32).T).astype(
                        BF16_NP
                    )
                ),
                "bibo": bibo_c,
            }
        )

    _ensure_axon_hooks_module()
    nc = _get_nc(cap)
    res = run_bass_kernel_spmd(
        nc, in_maps, core_ids=list(range(N_CORES)), trace=_trace
    )

    # Unshard: scatter-add each core's expert rows scaled by its fp32
    # routing weight.
    out = np.zeros((T, H), dtype=np.float32)
    for c in range(N_CORES):
        tl = tok_lists[c]
        n = len(tl)
        # out param is [128, KH*cap] block-major: token t0+t of block ib at
        # [p, KH*t0 + k*b + t] -> rows h = k*128+p
        o = np.asarray(res.results[c]["out"]).astype(np.float32)  # [P, KH*cap]
        o_rows = np.empty((n, H), dtype=np.float32)
        t0 = 0
        for b in blocks:
            if t0 >= n:
                break
            m = min(b, n - t0)
            blk = o[:, KH * t0 : KH * (t0 + b)].reshape(P, KH, b)
            # [P, KH, b] -> tokens t0..t0+m, features h=k*128+p
            o_rows[t0 : t0 + m] = blk[:, :, :m].transpose(2, 1, 0).reshape(m, H)
            t0 += b
        out[tl] += w_lists[c][:n, None] * o_rows
    out = out.reshape(x.shape)
    if _trace:
        return out, res
    return out
